# revision 7
# baseline (speedup 1.0000x reference)
"""Multi-head GAT layer for Trainium2 — 8 heads sharded across 8 NeuronCores.

Per head h (N=4096 nodes, F=64 features):
    ltg   = graph @ W[h]                          [N, F]
    s     = ltg @ a_src,  d = ltg @ a_dst         [N]
    E     = leaky_relu(s[:, None] + d[None, :], 0.2)
    Alpha = softmax(E, axis=-1)
    out   = Alpha @ ltg

Key algebraic trick used on-device: with z = s_i + d_j and
M_ij = [z >= 0],

    exp(leaky_relu(z)) = M_ij * e^{s_i} e^{d_j} + (1-M_ij) * e^{0.2 s_i} e^{0.2 d_j}

so the whole N x N softmax reduces to mask generation plus masked
matmuls on the PE, with the (1-M) branch handled via total-minus-masked
(identical quantized summands keep the two exactly consistent):

    num_i / e^{0.2 s_i} = r_i * (M @ R1)_i + (T2 - (M @ R2))_i
    den_i / e^{0.2 s_i} = r_i * (M @ v)_i  + (t2 - (M @ v2)_i)
    out_i = num_i / den_i            with r_i = e^{0.8 s_i}

R1 = v .* ltg, R2 = v2 .* ltg, v = e^d, v2 = e^{0.2 d}.

Perf structure vs the bf16 baseline:
  * masks and R are fp8e4 (TRN E4M3): FWL loads fp8 weights 4 els/cycle,
    so the 1024 mask-stationary matmuls are stream-bound (~130 cyc),
    not weight-load-bound.
  * mask generation is split across DVE (is_ge), ACT (saturated
    sigmoid) and GpSimd (is_ge) — three engines instead of one.
  * the graph projection runs in fp16 (1 cyc/row vs fp32's 4).
  * epilogue is pure DVE/Pool (scalar_tensor_tensor fusion), no ACT.

Heads are fully independent: core h computes head h; no collectives.
"""

import os
from contextlib import ExitStack

import numpy as np

N, F_IN, F, H = 4096, 64, 64, 8
P = 128
NB = N // P           # 32 node blocks
ISUP = 4              # i-blocks per PSUM super-block (4 banks of accumulators)
NSUP = NB // ISUP     # 8 super iterations
RC = 130              # R columns per j-block: R1(64) | R2(64) | v | v2
SIG_SCALE = 65536.0

# mask-engine split per 32 j-blocks: counts for (DVE, ACT, Pool)
MV = int(os.environ.get("GAT_MV", "22"))
MA = int(os.environ.get("GAT_MA", "10"))
MP = int(os.environ.get("GAT_MP", "0"))
assert MV + MA + MP == 32
_CACHE = {}


def _build():
    import concourse.bass as bass  # noqa: F401
    import concourse.mybir as mybir
    import concourse.tile as tile
    from concourse import bacc

    dt = mybir.dt
    f32 = dt.float32
    f16 = dt.float16
    bf16 = dt.bfloat16
    f8 = dt.float8e4
    Alu = mybir.AluOpType
    Act = mybir.ActivationFunctionType

    nc = bacc.Bacc("TRN2", debug=False, num_devices=H)
    graph_d = nc.dram_tensor("graph", [N, F_IN], f32, kind="ExternalInput").ap()
    w_d = nc.dram_tensor("w", [F_IN, F], f32, kind="ExternalInput").ap()
    a_d = nc.dram_tensor("a", [2, F], f32, kind="ExternalInput").ap()
    out_d = nc.dram_tensor("out", [N, F], f32, kind="ExternalOutput").ap()

    ident16_d = nc.inline_tensor(np.eye(P, dtype=np.float16), name="ident16")
    ident32_d = nc.inline_tensor(np.eye(F_IN, dtype=np.float32), name="ident32")

    # engine for mask tile (sup, b): fixed per-b pattern
    mask_eng = (["v"] * MV + ["a"] * MA + ["p"] * MP)

    with tile.TileContext(nc) as tc, ExitStack() as ctx:
        persist = ctx.enter_context(tc.tile_pool(name="persist", bufs=1))
        sps = ctx.enter_context(tc.tile_pool(name="sps", bufs=2, space="PSUM"))
        accp = ctx.enter_context(tc.tile_pool(name="acc", bufs=1, space="PSUM"))
        ssb = ctx.enter_context(tc.tile_pool(name="ssb", bufs=4))
        gp = ctx.enter_context(tc.tile_pool(name="gp", bufs=6))
        g16p = ctx.enter_context(tc.tile_pool(name="g16p", bufs=4))
        mp = ctx.enter_context(tc.tile_pool(name="mask", bufs=3))
        ep = ctx.enter_context(tc.tile_pool(name="ep", bufs=6))

        ident16 = persist.tile([P, P], f16)
        nc.sync.dma_start(ident16[:], ident16_d.ap())
        ident32 = persist.tile([F_IN, F_IN], f32)
        nc.sync.dma_start(ident32[:], ident32_d.ap())
        ones_row_bf = persist.tile([1, P], bf16)
        nc.vector.memset(ones_row_bf[:], 1.0)
        ones_row_f = persist.tile([1, P], f32)
        nc.vector.memset(ones_row_f[:], 1.0)
        ones_col_bf = persist.tile([P, 1], bf16)
        nc.vector.memset(ones_col_bf[:], 1.0)

        wf = persist.tile([F_IN, F], f32)
        nc.sync.dma_start(wf[:], w_d[:])
        a2_sb = persist.tile([F, 2], f32)
        nc.sync.dma_start(a2_sb[:], a_d.rearrange("t k -> k t"))

        # fused fp16 [W | w_s | w_d] for the projection matmul
        w16 = persist.tile([F_IN, F + 2], f16)
        nc.gpsimd.tensor_copy(w16[:, 0:F], wf[:])

        gT16 = persist.tile([F_IN, N], f16)           # graph^T fp16
        ltgsd = persist.tile([P, 66 * NB], f32)       # per b: ltg (64) | s | d
        negd = persist.tile([P, NB], f32)             # -d columns
        dscaled = persist.tile([P, NB], f32)          # SIG_SCALE * d
        rcol = persist.tile([P, NB], f32)             # e^{0.8 s}
        vcol = persist.tile([P, NB], f32)             # e^d
        v2col = persist.tile([P, NB], f32)            # e^{0.2 d}
        sdrow = persist.tile([2, N], bf16)            # s, d rows (bcast feed)
        s_rep = persist.tile([P, N], bf16)            # s broadcast down partitions
        r_all = persist.tile([P, RC * NB], bf16)      # [R1|R2|v|v2] per b
        t2acc = persist.tile([1, 66], f32)            # T2 row (SBUF copy)
        t2rep = persist.tile([P, 66], f32)            # T2 bcast down partitions
        eps_all = persist.tile([P, 130 * NB], f32)    # psum snapshots per i-block

        ltgsd_v = ltgsd.rearrange("p (b c) -> p b c", c=66)
        r_v = r_all.rearrange("p (b c) -> p b c", c=RC)
        eps_v = eps_all.rearrange("p (b c) -> p b c", c=130)

        # [w_s | w_d] = (W^T).T @ a2, via a small f32 transpose
        wT_ps = sps.tile([F, F_IN], f32, tag="tp")
        nc.tensor.transpose(wT_ps[:], wf[:], ident32[:])
        wT_sb = ssb.tile([F, F_IN], f32)
        nc.vector.tensor_copy(wT_sb[:], wT_ps[:])
        wsd_ps = sps.tile([F_IN, 2], f32, tag="pj")
        nc.tensor.matmul(wsd_ps[:], wT_sb[:], a2_sb[:])
        nc.scalar.copy(w16[:, F:F + 2], wsd_ps[:])

        mask_tiles = {}

        def emit_mask(sup, b, allow_act=True):
            eng = mask_eng[b]
            if eng == "a" and not allow_act:
                eng = "v"
            i0 = sup * ISUP * P
            mt = mp.tile([P, ISUP * P], bf16, tag=f"m{b}", name=f"mask{b}")
            if eng == "a":
                nc.scalar.activation(
                    mt[:], s_rep[:, i0:i0 + ISUP * P], Act.Sigmoid,
                    bias=dscaled[:, b:b + 1], scale=SIG_SCALE)
            elif eng == "v":
                nc.vector.tensor_scalar(
                    mt[:], s_rep[:, i0:i0 + ISUP * P],
                    negd[:, b:b + 1], None, op0=Alu.is_ge)
            else:
                nc.gpsimd.tensor_scalar(
                    mt[:], s_rep[:, i0:i0 + ISUP * P],
                    negd[:, b:b + 1], None, op0=Alu.is_ge)
            mask_tiles[(sup, b)] = mt

        def do_group(g):
            """s/d-derived tables + R blocks for blocks 4g..4g+3."""
            bsl = slice(4 * g, 4 * g + 4)
            d_src = ltgsd_v[:, bsl, 65]     # [128, 4] strided
            s_src = ltgsd_v[:, bsl, 64]
            nc.gpsimd.tensor_scalar(negd[:, bsl], d_src, -1.0, None,
                                    op0=Alu.mult)
            nc.gpsimd.tensor_scalar(dscaled[:, bsl], d_src, SIG_SCALE, None,
                                    op0=Alu.mult)
            nc.scalar.activation(rcol[:, bsl], s_src, Act.Exp, scale=0.8)
            nc.scalar.activation(vcol[:, bsl], d_src, Act.Exp)
            nc.scalar.activation(v2col[:, bsl], d_src, Act.Exp, scale=0.2)
            for bb in range(4 * g, 4 * g + 4):
                ltg_b = ltgsd[:, 66 * bb:66 * bb + F]
                r0 = RC * bb
                nc.vector.tensor_scalar(r_all[:, r0:r0 + F], ltg_b,
                                        vcol[:, bb:bb + 1], None, op0=Alu.mult)
                nc.scalar.mul(r_all[:, r0 + F:r0 + 2 * F], ltg_b,
                              v2col[:, bb:bb + 1])
            nc.gpsimd.tensor_copy(r_v[:, bsl, 128], vcol[:, bsl])
            nc.gpsimd.tensor_copy(r_v[:, bsl, 129], v2col[:, bsl])

        # pipelined setup over 32 blocks
        for b in range(NB):
            g_sb = gp.tile([P, F_IN], f32)
            nc.sync.dma_start(g_sb[:], graph_d[b * P:(b + 1) * P, :])
            g16 = g16p.tile([P, F_IN], f16)
            nc.gpsimd.tensor_copy(g16[:], g_sb[:])
            gT_ps = sps.tile([F_IN, P], f16, tag="tp")
            nc.tensor.transpose(gT_ps[:], g16[:], ident16[:])
            nc.vector.tensor_copy(gT16[:, b * P:(b + 1) * P], gT_ps[:])
            prj_ps = sps.tile([P, F + 2], f32, tag="pj")
            nc.tensor.matmul(prj_ps[:], gT16[:, b * P:(b + 1) * P], w16[:])
            nc.scalar.copy(ltgsd[:, 66 * b:66 * (b + 1)], prj_ps[:])
            if b % 4 == 3:
                c = b // 4
                srow_ps = sps.tile([2, 512], f32, tag="pj", name="srow_ps")
                nc.tensor.matmul(srow_ps[:], w16[:, F:F + 2],
                                 gT16[:, c * 512:(c + 1) * 512])
                nc.vector.tensor_copy(sdrow[:, c * 512:(c + 1) * 512],
                                      srow_ps[:])
                bc_ps = sps.tile([P, 512], f32, tag="tp", name="bc_ps")
                nc.tensor.matmul(bc_ps[:], ones_row_bf[:],
                                 sdrow[0:1, c * 512:(c + 1) * 512])
                nc.scalar.copy(s_rep[:, c * 512:(c + 1) * 512], bc_ps[:])
                g = c
                do_group(g)
                # prefill masks for early supers (sup k needs s_rep chunk k,
                # ready after setup block 4k+3); ACT is busy with exps here.
                for sup in range(min(3, g + 1)):
                    for bb in range(4 * g, 4 * g + 4):
                        emit_mask(sup, bb, allow_act=False)

        # T2 burst: transient psum tile, released right after the copy
        t2_ps = sps.tile([1, 66], f32, tag="pj", name="t2ps_g")
        for bb in range(NB):
            r0 = RC * bb
            nc.tensor.matmul(t2_ps[:], ones_col_bf[:],
                             r_all[:, r0 + F:r0 + 130],
                             start=(bb == 0), stop=(bb == NB - 1))
        nc.vector.tensor_copy(t2acc[:], t2_ps[:])
        t2rep_ps = sps.tile([P, 66], f32, tag="tp", name="t2rep_ps")
        nc.tensor.matmul(t2rep_ps[:], ones_row_f[:], t2acc[:])
        nc.scalar.copy(t2rep[:], t2rep_ps[:])

        # ---- main masked-matmul loop ----

        late = []

        def late_phase(sup):
            """Epilogue on SBUF snapshots; needs t2rep. DVE + Pool only."""
            i0 = sup * ISUP
            r_v4 = rcol[:, i0:i0 + ISUP]
            den1 = ep.tile([P, ISUP], f32, tag="den1", name="den1")
            nc.vector.tensor_tensor(den1[:], r_v4, eps_v[:, i0:i0 + ISUP, 128],
                                    op=Alu.mult)
            dd = ep.tile([P, ISUP], f32, tag="dd", name="dd")
            nc.vector.tensor_tensor(dd[:], t2rep[:, 65:66].to_broadcast([P, ISUP]),
                                    eps_v[:, i0:i0 + ISUP, 129], op=Alu.subtract)
            den = ep.tile([P, ISUP], f32, tag="den", name="den")
            nc.vector.tensor_tensor(den[:], den1[:], dd[:], op=Alu.add)
            rden = ep.tile([P, ISUP], f32, tag="rden", name="rden")
            nc.vector.reciprocal(rden[:], den[:])
            for t in range(ISUP):
                i = i0 + t
                e0 = 130 * i
                # scalar_tensor_tensor is DVE-only; Pool takes the tail ops
                eng = nc.gpsimd
                A = ep.tile([P, F], f32, tag="nA", name="nA")
                nc.vector.scalar_tensor_tensor(
                    A[:], eps_all[:, e0:e0 + F], rcol[:, i:i + 1],
                    eps_all[:, e0 + F:e0 + 2 * F],
                    op0=Alu.mult, op1=Alu.subtract)
                num = ep.tile([P, F], f32, tag="num", name="num")
                eng.tensor_tensor(num[:], A[:], t2rep[:, 0:F], op=Alu.add)
                ot = ep.tile([P, F], f32, tag="ot", name="ot")
                eng.tensor_scalar(ot[:], num[:], rden[:, t:t + 1], None,
                                  op0=Alu.mult)
                nc.sync.dma_start(out_d[i * P:(i + 1) * P, :], ot[:])

        for sup in range(NSUP):
            for b in range(NB):
                if (sup, b) not in mask_tiles:
                    emit_mask(sup, b)
            mtiles = [mask_tiles.pop((sup, b)) for b in range(NB)]

            acc = accp.tile([P, 512 * ISUP], f32, name="acc")
            for b in range(NB):
                r0 = RC * b
                for t in range(ISUP):
                    nc.tensor.matmul(
                        acc[:, 512 * t:512 * t + RC],
                        mtiles[b][:, t * P:(t + 1) * P],
                        r_all[:, r0:r0 + RC],
                        start=(b == 0), stop=(b == NB - 1))
            # snapshot psum -> SBUF (releases the accumulator quickly)
            for t in range(ISUP):
                i = sup * ISUP + t
                nc.vector.tensor_copy(eps_all[:, 130 * i:130 * (i + 1)],
                                      acc[:, 512 * t:512 * t + 130])
            late.append(sup)
            if len(late) > 1:
                late_phase(late.pop(0))
        for sup in late:
            late_phase(sup)

    nc.compile()
    return nc


def _get_nc():
    if "nc" not in _CACHE:
        _CACHE["nc"] = _build()
    return _CACHE["nc"]


def kernel(graph, W, a):
    from concourse.bass_utils import run_bass_kernel_spmd

    graph = np.ascontiguousarray(np.asarray(graph, dtype=np.float32))
    W = np.asarray(W, dtype=np.float32)
    a = np.asarray(a, dtype=np.float32)

    nc = _get_nc()
    in_maps = [
        {
            "graph": graph,
            "w": np.ascontiguousarray(W[h]),
            "a": np.ascontiguousarray(a[h].reshape(2, F)),
        }
        for h in range(H)
    ]
    trace = bool(int(os.environ.get("GAT_TRACE", "0")))
    res = run_bass_kernel_spmd(nc, in_maps, core_ids=list(range(H)), trace=trace)
    _CACHE["last_result"] = res
    return np.stack([res.results[h]["out"] for h in range(H)], axis=0)
